# revision 12
# baseline (speedup 1.0000x reference)
"""Trainium2 Bass kernel for a pre-LN transformer block (B=128,T=256,C=384,H=6,D=64).

Strategy: pure data-parallel over the batch dim across 8 NeuronCores (16
batches/core). All matmuls run as float32r (fp32 storage, full PE rate for
moving dim >= 256). LayerNorm gamma/beta are folded into the projection
weights on the host, so the device LN is just (x - mu) * rstd with
bn_stats/bn_aggr statistics. Attention runs in the transposed-score
orientation (scores^T = k q^T with [S,T] layout) so softmax denominators
come from an all-ones matmul that simultaneously broadcasts them across
partitions; causal masking is done by zeroing exp(scores) with
gpsimd.affine_select; normalization is deferred to the PSUM->SBUF
evacuation of attn@v.
"""

import sys

if "/opt/trn_rl_repo" not in sys.path:
    sys.path.insert(0, "/opt/trn_rl_repo")

import numpy as np

import concourse.bass as bass
import concourse.mybir as mybir
import concourse.tile as tile
from concourse import bacc

F32 = mybir.dt.float32
F32R = mybir.dt.float32r
AF = mybir.ActivationFunctionType
ALU = mybir.AluOpType

B, T, C, H, D = 128, 256, 384, 6, 64
NCORES = 8
BL = B // NCORES          # batches per core
F = 4 * C                 # 1536
P = 128
TCH = T // P              # 2 token chunks
CCH = C // P              # 3 channel chunks
FCH = F // P              # 12 ffn chunks
HD = H * D                # 384
SCALE = float(C) ** -0.5  # reference scales by full model dim
EPS = 1e-5


def build_program(bl=BL, flags=frozenset()):
    """Build the per-core Bass program. `flags` lists which bias terms are
    nonzero ('qb','kb','vb','bo','b1','b2') so zero biases cost nothing."""
    use_qb = "qb" in flags
    use_kb = "kb" in flags
    use_vb = "vb" in flags
    use_bo = "bo" in flags
    use_b1 = "b1" in flags
    use_b2 = "b2" in flags

    nc = bacc.Bacc("TRN2", target_bir_lowering=False, debug=False,
                   num_devices=NCORES)

    x_d = nc.dram_tensor("x", [bl, T, C], F32, kind="ExternalInput")
    wq_d = nc.dram_tensor("wq", [P, CCH, HD], F32R, kind="ExternalInput")
    wk_d = nc.dram_tensor("wk", [P, CCH, HD], F32R, kind="ExternalInput")
    wv_d = nc.dram_tensor("wv", [P, CCH, HD], F32R, kind="ExternalInput")
    qb_d = nc.dram_tensor("qb", [P, CCH], F32, kind="ExternalInput")
    kb_d = nc.dram_tensor("kb", [P, CCH], F32, kind="ExternalInput")
    vb_d = nc.dram_tensor("vb", [1, HD], F32R, kind="ExternalInput")
    wo_d = nc.dram_tensor("wo", [D, H, C], F32R, kind="ExternalInput")
    bo_d = nc.dram_tensor("bo", [1, C], F32R, kind="ExternalInput")
    w1_d = nc.dram_tensor("w1", [P, CCH, F], F32R, kind="ExternalInput")
    b1_d = nc.dram_tensor("b1c", [P, FCH], F32, kind="ExternalInput")
    w2_d = nc.dram_tensor("w2", [P, FCH, C], F32R, kind="ExternalInput")
    b2_d = nc.dram_tensor("b2", [1, C], F32R, kind="ExternalInput")
    id_d = nc.dram_tensor("ident", [P, P], F32R, kind="ExternalInput")
    on_d = nc.dram_tensor("onesm", [P, P], F32R, kind="ExternalInput")
    y_d = nc.dram_tensor("y", [bl, T, C], F32, kind="ExternalOutput")

    with tile.TileContext(nc) as tc:
        with (
            tc.tile_pool(name="wpool", bufs=1) as wp,
            tc.tile_pool(name="work", bufs=2) as wk_pool,
            tc.tile_pool(name="big", bufs=1) as bigp,
            tc.tile_pool(name="ps_big", bufs=2, space="PSUM") as psb,
            tc.tile_pool(name="ps_att", bufs=3, space="PSUM") as psa,
            tc.tile_pool(name="ps_dp", bufs=1, space="PSUM") as psd,
        ):
            # ---- load weights/constants once ----
            wq = wp.tile([P, CCH, HD], F32R)
            wkk = wp.tile([P, CCH, HD], F32R)
            wv = wp.tile([P, CCH, HD], F32R)
            wo = wp.tile([D, H, C], F32R)
            w1 = wp.tile([P, CCH, F], F32R)
            w2 = wp.tile([P, FCH, C], F32R)
            ident = wp.tile([P, P], F32R)
            ones_t = wp.tile([P, P], F32R)
            epsb = wp.tile([P, 1], F32)
            nc.gpsimd.memset(epsb[:], EPS)
            nc.sync.dma_start(wq[:], wq_d[:])
            nc.sync.dma_start(wkk[:], wk_d[:])
            nc.sync.dma_start(wv[:], wv_d[:])
            nc.sync.dma_start(wo[:], wo_d[:])
            nc.sync.dma_start(w1[:], w1_d[:])
            nc.sync.dma_start(w2[:], w2_d[:])
            nc.sync.dma_start(ident[:], id_d[:])
            nc.sync.dma_start(ones_t[:], on_d[:])
            qb = kb = vb = bo = b1c = b2 = None
            if use_qb:
                qb = wp.tile([P, CCH], F32)
                nc.sync.dma_start(qb[:], qb_d[:])
            if use_kb:
                kb = wp.tile([P, CCH], F32)
                nc.sync.dma_start(kb[:], kb_d[:])
            if use_vb:
                vb = wp.tile([1, HD], F32R)
                nc.sync.dma_start(vb[:], vb_d[:])
            if use_bo:
                bo = wp.tile([1, C], F32R)
                nc.sync.dma_start(bo[:], bo_d[:])
            if use_b1:
                b1c = wp.tile([P, FCH], F32)
                nc.sync.dma_start(b1c[:], b1_d[:])
            if use_b2:
                b2 = wp.tile([1, C], F32R)
                nc.sync.dma_start(b2[:], b2_d[:])

            def layer_norm_T(src, evac_act):
                """src: [P, TCH, C] sbuf tile (token-major). Returns [P, CCH, T]
                sbuf tile holding (src - mu) * rstd transposed."""
                st6 = wk_pool.tile([P, TCH, 6], F32, tag="st6")
                mv = wk_pool.tile([P, TCH, 2], F32, tag="mv")
                rstd = wk_pool.tile([P, TCH], F32, tag="rstd")
                for tch in range(TCH):
                    nc.vector.bn_stats(st6[:, tch, :], src[:, tch, :])
                    nc.vector.bn_aggr(mv[:, tch, :], st6[:, tch, :])
                # rstd = exp(-0.5 * ln(var + eps))
                nc.scalar.activation(rstd[:], mv[:, :, 1], AF.Ln, bias=epsb[:])
                nc.scalar.activation(rstd[:], rstd[:], AF.Exp, scale=-0.5)
                xn = wk_pool.tile([P, TCH, C], F32R, tag="xn")
                for tch in range(TCH):
                    nc.vector.tensor_scalar(
                        xn[:, tch, :], src[:, tch, :],
                        mv[:, tch, 0:1], rstd[:, tch:tch + 1],
                        ALU.subtract, ALU.mult,
                    )
                tr = psb.tile([P, CCH, T], F32R, tag="big")
                for tch in range(TCH):
                    for cc in range(CCH):
                        nc.tensor.transpose(
                            tr[:, cc, tch * P:(tch + 1) * P],
                            xn[:, tch, cc * P:(cc + 1) * P],
                            ident[:],
                        )
                xnT = wk_pool.tile([P, CCH, T], F32R, tag="xnT")
                if evac_act:
                    nc.scalar.copy(xnT[:], tr[:])
                else:
                    nc.vector.tensor_copy(xnT[:], tr[:])
                return xnT

            for b in range(bl):
                # ---- load x ----
                xt = wk_pool.tile([P, TCH, C], F32, tag="xt")
                nc.sync.dma_start(
                    xt[:], x_d[b].rearrange("(tc p) c -> p tc c", p=P))

                # ---- LN1 -> xnT [c, t] ----
                xnT = layer_norm_T(xt, evac_act=True)

                # ---- q,k transposed: [hd, t]; v natural: [s, hd] ----
                qsb = wk_pool.tile([P, CCH, T], F32R, tag="qsb")
                ksb = wk_pool.tile([P, CCH, T], F32R, tag="ksb")
                vsb = wk_pool.tile([P, TCH, HD], F32R, tag="vsb")
                for name, wmat, bias_t, use_b, dst, eng in (
                    ("q", wq, qb, use_qb, qsb, "act"),
                    ("k", wkk, kb, use_kb, ksb, "act"),
                ):
                    pp = psb.tile([P, CCH, T], F32, tag="big")
                    for mc in range(CCH):
                        for kc in range(CCH):
                            nc.tensor.matmul(
                                pp[:, mc, :],
                                (wmat[:, kc, mc * P:(mc + 1) * P]),
                                (xnT[:, kc, :]),
                                start=(kc == 0), stop=(kc == CCH - 1),
                            )
                    if use_b:
                        for mc in range(CCH):
                            nc.scalar.activation(
                                dst[:, mc, :], pp[:, mc, :], AF.Identity,
                                bias=bias_t[:, mc:mc + 1])
                    elif eng == "act":
                        nc.scalar.copy(dst[:], pp[:])
                    else:
                        nc.vector.tensor_copy(dst[:], pp[:])

                vp = psb.tile([P, TCH, 512], F32, tag="big")
                for sc in range(TCH):
                    for kc in range(CCH):
                        nc.tensor.matmul(
                            vp[:, sc, :HD],
                            (xnT[:, kc, sc * P:(sc + 1) * P]),
                            (wv[:, kc, :]),
                            start=(kc == 0), stop=(kc == CCH - 1 and not use_vb),
                        )
                    if use_vb:
                        nc.tensor.matmul(
                            vp[:, sc, :HD], (ones_t[0:1, :]), (vb[0:1, :]),
                            start=False, stop=True)
                nc.vector.tensor_copy(vsb[:], vp[:, :, :HD])

                # ---- attention: e_all[s%128, schunk, h, t] ----
                e_all = bigp.tile([P, TCH, H, T], F32R, tag="e_all")
                rbc = bigp.tile([P, H, T], F32, tag="rbc")
                osb = wk_pool.tile([64, H, T], F32R, tag="osb")
                for h in range(H):
                    hc, ho = h // 2, 64 * (h % 2)
                    sp = psa.tile([P, TCH, T], F32, tag="att")
                    for sc in range(TCH):
                        nc.tensor.matmul(
                            sp[:, sc, :],
                            (ksb[ho:ho + D, hc, sc * P:(sc + 1) * P]),
                            (qsb[ho:ho + D, hc, :]),
                            start=True, stop=True,
                        )
                    nc.scalar.activation(
                        e_all[:, :, h, :], sp[:], AF.Exp, scale=SCALE)
                    for sc in range(TCH):
                        # keep where t - s >= 0, else 0
                        nc.gpsimd.affine_select(
                            out=e_all[:, sc, h, :], in_=e_all[:, sc, h, :],
                            pattern=[[1, T]], compare_op=ALU.is_ge,
                            fill=0.0, base=-(sc * P), channel_multiplier=-1,
                        )
                for pc in range(H // 2):
                    dp = psd.tile([P, 2, T], F32, tag="dp")
                    ops = []
                    for i in range(2):
                        h = 2 * pc + i
                        for sc in range(TCH):
                            nc.tensor.matmul(
                                dp[:, i, :], (ones_t[:]),
                                (e_all[:, sc, h, :]),
                                start=(sc == 0), stop=(sc == TCH - 1))
                        op_h = psa.tile([D, T], F32, tag="att")
                        ops.append(op_h)
                        for sc in range(TCH):
                            nc.tensor.matmul(
                                op_h[:, :],
                                (vsb[:, sc, h * D:(h + 1) * D]),
                                (e_all[:, sc, h, :]),
                                start=(sc == 0), stop=(sc == TCH - 1))
                    nc.vector.reciprocal(rbc[:, 2 * pc:2 * pc + 2, :], dp[:])
                    for i in range(2):
                        h = 2 * pc + i
                        nc.vector.tensor_tensor(
                            osb[:, h, :],
                            ops[i][:, :],
                            rbc[0:D, h, :],
                            ALU.mult,
                        )

                # ---- out proj + residual ----
                ap_t = psb.tile([P, TCH, 512], F32, tag="big")
                for tcc in range(TCH):
                    for h in range(H):
                        nc.tensor.matmul(
                            ap_t[:, tcc, :C],
                            (osb[:, h, tcc * P:(tcc + 1) * P]),
                            (wo[:, h, :]),
                            start=(h == 0), stop=(h == H - 1 and not use_bo))
                    if use_bo:
                        nc.tensor.matmul(
                            ap_t[:, tcc, :C], (ones_t[0:1, :]), (bo[0:1, :]),
                            start=False, stop=True)
                xnew = wk_pool.tile([P, TCH, C], F32, tag="xnew")
                for tcc in range(TCH):
                    nc.vector.tensor_tensor(
                        xnew[:, tcc, :], ap_t[:, tcc, :C], xt[:, tcc, :], ALU.add)

                # ---- LN2 -> xn2T ----
                xn2T = layer_norm_T(xnew, evac_act=False)

                # ---- FFN1: hT [f, t] with relu ----
                hsb = bigp.tile([P, FCH, T], F32R, tag="hsb")
                for mo2 in range(FCH // 2):
                    hp = psa.tile([P, 2, T], F32, tag="att")
                    for i in range(2):
                        mo = 2 * mo2 + i
                        for kc in range(CCH):
                            nc.tensor.matmul(
                                hp[:, i, :],
                                (w1[:, kc, mo * P:(mo + 1) * P]),
                                (xn2T[:, kc, :]),
                                start=(kc == 0), stop=(kc == CCH - 1))
                    if use_b1:
                        for i in range(2):
                            mo = 2 * mo2 + i
                            nc.scalar.activation(
                                hsb[:, mo, :], hp[:, i, :], AF.Relu,
                                bias=b1c[:, mo:mo + 1])
                    else:
                        nc.scalar.activation(
                            hsb[:, 2 * mo2:2 * mo2 + 2, :], hp[:], AF.Relu)

                # ---- FFN2 + residual ----
                fp = psb.tile([P, TCH, 512], F32, tag="big")
                for tcc in range(TCH):
                    for ko in range(FCH):
                        nc.tensor.matmul(
                            fp[:, tcc, :C],
                            (hsb[:, ko, tcc * P:(tcc + 1) * P]),
                            (w2[:, ko, :]),
                            start=(ko == 0), stop=(ko == FCH - 1 and not use_b2))
                    if use_b2:
                        nc.tensor.matmul(
                            fp[:, tcc, :C], (ones_t[0:1, :]), (b2[0:1, :]),
                            start=False, stop=True)
                yout = wk_pool.tile([P, TCH, C], F32, tag="yout")
                for tcc in range(TCH):
                    nc.vector.tensor_tensor(
                        yout[:, tcc, :], fp[:, tcc, :C], xnew[:, tcc, :], ALU.add)
                nc.sync.dma_start(
                    y_d[b].rearrange("(tc p) c -> p tc c", p=P), yout[:])

    nc.compile()
    return nc


def prep_weights(Wq, Wk, Wv, Wo, bo, W1, b1, W2, b2, g1, be1, g2, be2):
    """Fold LN gamma/beta into projection weights; rearrange to SBUF layouts."""
    f32 = np.float32

    def kchunk(w, kdim):  # [K, M] -> [P, K//P, M]
        m = w.shape[1]
        return np.ascontiguousarray(
            w.reshape(kdim // P, P, m).transpose(1, 0, 2)).astype(f32)

    Wq2 = Wq.transpose(1, 0, 2).reshape(C, HD)
    Wk2 = Wk.transpose(1, 0, 2).reshape(C, HD)
    Wv2 = Wv.transpose(1, 0, 2).reshape(C, HD)
    out = {
        "wq": kchunk(g1[:, None] * Wq2, C),
        "wk": kchunk(g1[:, None] * Wk2, C),
        "wv": kchunk(g1[:, None] * Wv2, C),
        "wo": np.ascontiguousarray(
            Wo.reshape(H, D, C).transpose(1, 0, 2)).astype(f32),
        "w1": kchunk(g2[:, None] * W1, C),
        "w2": kchunk(W2, F),
        "ident": np.eye(P, dtype=f32),
        "onesm": np.ones((P, P), dtype=f32),
    }
    qb = be1 @ Wq2
    kb = be1 @ Wk2
    vb = be1 @ Wv2
    b1e = be2 @ W1 + b1
    out["qb"] = np.ascontiguousarray(qb.reshape(CCH, P).T).astype(f32)
    out["kb"] = np.ascontiguousarray(kb.reshape(CCH, P).T).astype(f32)
    out["vb"] = vb[None, :].astype(f32)
    out["bo"] = bo[None, :].astype(f32)
    out["b1c"] = np.ascontiguousarray(b1e.reshape(FCH, P).T).astype(f32)
    out["b2"] = b2[None, :].astype(f32)
    flags = set()
    for name, vec in (("qb", qb), ("kb", kb), ("vb", vb),
                      ("bo", bo), ("b1", b1e), ("b2", b2)):
        if np.any(vec != 0):
            flags.add(name)
    return out, frozenset(flags)


_PROGRAM_CACHE = {}


def _get_program(bl, flags):
    key = (bl, flags)
    if key not in _PROGRAM_CACHE:
        _PROGRAM_CACHE[key] = build_program(bl, flags)
    return _PROGRAM_CACHE[key]


def kernel(x, Wq, Wk, Wv, Wo, bo, W1, b1, W2, b2, g1, be1, g2, be2, **kw):
    from concourse.bass_utils import run_bass_kernel_spmd

    args = [np.asarray(a, dtype=np.float32) for a in
            (x, Wq, Wk, Wv, Wo, bo, W1, b1, W2, b2, g1, be1, g2, be2)]
    x = args[0]
    wmap, flags = prep_weights(*args[1:])
    nc = _get_program(BL, flags)
    xs = x.reshape(NCORES, BL, T, C)
    in_maps = []
    for c in range(NCORES):
        m = {"x": np.ascontiguousarray(xs[c])}
        m.update(wmap)
        in_maps.append(m)
    res = run_bass_kernel_spmd(nc, in_maps, list(range(NCORES)), **kw)
    global _last_results
    _last_results = res
    y = np.stack([res.results[i]["y"] for i in range(NCORES)], axis=0)
    return y.reshape(B, T, C)


_last_results = None
